# revision 90
# baseline (speedup 1.0000x reference)
"""MetaOptNet SVM-CS head on 8 Trainium2 NeuronCores.

Math: the reference runs a 15-iteration Mehrotra interior-point solve of the
Crammer-Singer dual QP per task. Empirically (f64 replication) the IPM is
fully converged by iteration 15, so the target equals the QP optimum. We
compute that optimum with a fixed-matrix ADMM:

    per task:  K = S S^T  (25x25 Gram)
               W~ = rho * (K + (1+rho) I)^{-1}   (Newton-Schulz, 2 iters:
                   X1 = 2c I - c^2 H is the closed form of the first
                   iteration from X0 = c I, so only the second runs on the
                   PE, in fp32, squaring the ~1% linearization error away;
                   |I - cH| <= ~0.1 since 9 <= eig(K+9I) <= ~17)
               8x ADMM (rho=8), in (d1 = u-y, oy = y+oh/rho) state form:
                   t = center_ways(W~ @ d1) + oy
                   d1' = min(t, 2h - t);  oy' = max(t - (h - oh/rho), oh/rho)
                   where h = (C + 1/rho) oh; the last iteration stops after
                   x = center_ways(W~ @ d1)
               logits = (S Q^T)^T x * scale

The equality constraint A z = 0 (sum over ways per sample) reduces to
centering across ways because A A^T = n_way I; the KKT matrix is way-block-
diagonal with identical blocks K + (1+rho)I, which is what makes the single
25x25 inverse per task sufficient.

Sharding: pure data parallel, 16 tasks per core. Host-side work is layout
only (shard, transpose packing into 128-partition DMA tiles, one-hot
constants); all FLOPs run on-device. All inputs ride in two DRAM tensors
(one bf16 payload, one f32 const pack) to minimize per-dispatch buffer
bindings.

Precision: the QP (Gram, inverse, ADMM) runs in fp32. S^T/Q^T are shipped
and contracted in bf16, and the ADMM runs 8 iterations — together ~8.0e-3
max relative error on the logits (tolerance 2e-2). The epilogue contracts
C^T = Q S^T over d (both operands d-major, so S is shipped once, and the
flipped orientation streams only 25 rhs columns per matmul), then
transposes back via exact bf16 identity matmuls; HBM traffic is the
irreducible S + Q + consts (~8.2 MB/core) and the kernel sits at ~21.5us
cost-model time per core.

Tasks sit in 32-aligned 25-row partition blocks (PE tile_position constraint),
four tasks per 128-partition tile; the zero padding rides through every
matmul/elementwise op harmlessly.

PSUM discipline (hardware-verified): matmul start=True clears the written
partitions' has_written bits across a whole zero region of the bank, so
concurrent accumulation chains may share a bank ONLY on disjoint partitions
(Gram/C layouts), and single-shot chains sharing a bank's columns must be
one accumulation group (stage-5 logits).
"""

import sys

sys.path.insert(0, "/opt/trn_rl_repo")

from contextlib import ExitStack

import numpy as np

import concourse.bass as bass
import concourse.tile as tile
from concourse import mybir
from concourse.alu_op_type import AluOpType
from concourse.bass_utils import run_bass_kernel_spmd
from concourse.tile import TileContext

# ---------------------------------------------------------------------------
# Problem constants (hardcoded per the harness contract)
N_CORES = 8
B_TOT = 128
T = 16            # tasks per core
NS = 25           # support samples per task
NW = 5            # ways
NQ = 75           # queries per task
D = 2560          # feature dim
NCH = D // 128    # 20 d-chunks
G = 4             # task groups per core (4 tasks each -> 100-partition tiles)
GP = T // G       # tasks per group
RHO = 8.0
NS_C = 0.065      # Newton-Schulz init scale for H = K + 9I
ADMM_ITERS = 8    # rel err 8.0e-3 at 8 iters (vs 3.5e-3 at 10), tol 2e-2
C_REG = 0.1

F32 = mybir.dt.float32
BF16 = mybir.dt.bfloat16

# Packed-input column offsets. All bf16 payloads (S^T chunks, S row-blocks,
# Q^T chunks) ride in one [128, DATA_COLS] tensor; all f32 constants in one
# [128, CPACK_COLS] tensor. One DRAM tensor per dtype keeps the per-dispatch
# buffer-binding count (and the host->device staging surface) minimal.
ST0 = 0                       # S^T: NCH blocks of T*NS cols
QT0 = ST0 + NCH * T * NS      # Q^T: NCH blocks of T*NQ cols
DATA_COLS = QT0 + NCH * T * NQ
OHC0, H20, HMO0, BD0, SCL0 = 0, 20, 40, 60, 188
I75_0 = 189                   # 75x75 identity (rows 0-74) for the C^T->C PE transpose
CPACK_COLS = I75_0 + 75


# ---------------------------------------------------------------------------
# The walrus build here encodes at most ONE sync-wait command per instruction
# (TPB_CTRL / S3_LW setupSyncWait raises "Too many sync wait commands").
# Tile's scheduler freely attaches several waits to one instruction, so after
# scheduling we split the excess onto NoOps inserted immediately before the
# instruction on the same engine — identical semantics, encodable waits.
def _split_waits(nc, max_waits=1):
    cnt = 0
    for blk in nc.m.functions[0].blocks:
        insns = blk.instructions
        idx = 0
        while idx < len(insns):
            ins = insns[idx]
            si = ins.sync_info
            waits = list(si.on_wait) if si and si.on_wait else []
            if len(waits) > max_waits:
                si.on_wait = waits[:max_waits]
                for w in waits[max_waits:]:
                    nop = mybir.InstNoOp(name=f"waitnop_{cnt}", ins=[], outs=[])
                    cnt += 1
                    nop.engine = ins.engine
                    nop.sync_info = mybir.SyncInfo(on_wait=[w], on_update=[])
                    nc.register_instruction(nop, overwrite=True)
                    insns.insert(idx, nop)
                    idx += 1
            idx += 1
    return cnt


# ---------------------------------------------------------------------------
def _build_program():
    nc = bass.Bass("TRN2", target_bir_lowering=False)

    data_d = nc.dram_tensor("data", [128, DATA_COLS], BF16, kind="ExternalInput")
    cpack_d = nc.dram_tensor("cpack", [128, CPACK_COLS], F32, kind="ExternalInput")
    out_d = nc.dram_tensor("out", [NQ, T * NW], F32, kind="ExternalOutput")

    with ExitStack() as ctx:
        tc = ctx.enter_context(TileContext(nc))
        st_pool = ctx.enter_context(tc.tile_pool(name="st", bufs=1))
        qt_pool = ctx.enter_context(tc.tile_pool(name="qt", bufs=NCH))
        consts = ctx.enter_context(tc.tile_pool(name="consts", bufs=1))
        mats = ctx.enter_context(tc.tile_pool(name="mats", bufs=12))
        state = ctx.enter_context(tc.tile_pool(name="state", bufs=10))
        wout = ctx.enter_context(tc.tile_pool(name="wout", bufs=4))
        ctbp = ctx.enter_context(tc.tile_pool(name="ctbp", bufs=T))

        # ---- loads --------------------------------------------------------
        # All f32 consts (ohc/h2/hmo + blockdiag) ride in ONE DMA — each DMA
        # instruction costs ~500ns of queue time regardless of size, so the
        # mini-tensors are batched. i2/nine/cib are scalar multiples of the
        # block-diagonal mask, derived on the (early-idle) DVE so they don't
        # wait behind the Act engine's one-time function table load; a tiny
        # dummy activation right after the cpk DMA absorbs that table load
        # while the input DMAs stream.
        # cpk first: small DMAs pay a fixed ~2us completion latency, and the
        # Newton-Schulz constants derived from it gate the whole QP chain.
        cpk_sb = consts.tile([128, BD0 + 128], F32, tag="cpk")
        nc.gpsimd.dma_start(out=cpk_sb, in_=cpack_d[:, 0 : BD0 + 128])
        # S^T next, split across THREE queues (the Act queue is idle until
        # the first Newton-Schulz copy ~6us in) — the Gram -> NS -> ADMM
        # serial chain starts from the last S^T byte, so this is the head of
        # the critical path.
        st_tile = st_pool.tile([128, NCH * T * NS], BF16, tag="st")
        CW = T * NS
        # six DMAs (two per queue) so early chunks' completion semaphores
        # fire early; the Gram chains below are ordered by these arrival
        # times so each chain's stop lands on an early-arrived chunk
        st_splits = [
            (0, 3, nc.sync), (3, 7, nc.sync),
            (7, 10, nc.gpsimd), (10, 13, nc.gpsimd),
            (13, 16, nc.scalar), (16, 20, nc.scalar),
        ]
        for lo, hi, eng in st_splits:
            eng.dma_start(
                out=st_tile[:, lo * CW : hi * CW],
                in_=data_d[:, ST0 + lo * CW : ST0 + hi * CW],
            )
        st_sb = [
            st_tile[:, c * T * NS : (c + 1) * T * NS] for c in range(NCH)
        ]
        # chunk order by estimated DMA completion (first split per queue,
        # then second splits, latest last-emitted first within the chain)
        ARR = [0, 1, 2, 13, 14, 15, 7, 8, 9, 3, 4, 5, 6, 16, 17, 18, 19,
               10, 11, 12]
        ohc_sb = cpk_sb[:, OHC0 : OHC0 + 20]
        h2_sb = cpk_sb[:, H20 : H20 + 20]
        hmo_sb = cpk_sb[:, HMO0 : HMO0 + 20]
        bd_sb = cpk_sb[:, BD0 : BD0 + 128]
        warm_sb = consts.tile([128, 1], F32, tag="warm")
        nc.scalar.activation(
            warm_sb, cpk_sb[:, 0:1], mybir.ActivationFunctionType.Copy
        )
        i2_sb = consts.tile([128, 128], F32, tag="i2")
        nc.vector.tensor_scalar_mul(i2_sb, bd_sb, 2.0)
        cbd2_sb = consts.tile([128, 128], F32, tag="cbd2")
        nc.vector.tensor_scalar_mul(cbd2_sb, bd_sb, 2.0 * NS_C)
        nine_sb = consts.tile([128, 128], F32, tag="nine")
        nc.vector.tensor_scalar_mul(nine_sb, bd_sb, 1.0 + RHO)
        scale_sb = consts.tile([NQ, 1], F32, tag="scale")
        nc.sync.dma_start(
            out=scale_sb,
            in_=cpack_d[0:1, SCL0 : SCL0 + 1].to_broadcast([NQ, 1]),
        )
        i75f_sb = consts.tile([NQ, NQ], F32, tag="i75f")
        nc.sync.dma_start(out=i75f_sb, in_=cpack_d[0:NQ, I75_0 : I75_0 + NQ])
        i75b_sb = consts.tile([NQ, NQ], BF16, tag="i75b")
        nc.vector.tensor_copy(i75b_sb, i75f_sb)

        # ADMM state: d1 = u - y (init ohc), oy = y + ohc (init ohc). oy
        # aliases the const tile; d1 is a bf16 copy (the xp matmul runs with
        # bf16 lhsT/rhs, so d1 stays bf16 across iterations).
        d1b0 = state.tile([128, 20], BF16, tag="d1b0")
        nc.vector.tensor_copy(d1b0, ohc_sb)
        d1_sb = d1b0
        oy_sb = ohc_sb

        # all Q^T chunks loaded up front, interleaved across both DMA queues
        qt_sb = []
        for c in range(NCH):
            t_ = qt_pool.tile([128, T * NQ], BF16, tag="qt")
            eng = nc.sync if c % 2 == 0 else nc.gpsimd
            eng.dma_start(
                out=t_, in_=data_d[:, QT0 + c * T * NQ : QT0 + (c + 1) * T * NQ]
            )
            qt_sb.append(t_)


        # ---- stage 1: K = S S^T, block-diagonal per 4-task group ----------
        h_all = []
        with tc.tile_pool(name="kpsum", bufs=4, space="PSUM") as kpsum:
            for g in range(G):
                kp = kpsum.tile([128, 128], F32, tag="kp")
                nc.vector.memset(kp, 0.0)
                for c in ARR:
                    for tp in range(GP):
                        t = g * GP + tp
                        sl = slice(tp * 32, tp * 32 + NS)
                        tsl = slice(t * NS, (t + 1) * NS)
                        nc.tensor.matmul(
                            kp[sl, sl],
                            lhsT=st_sb[c][:, tsl],
                            rhs=st_sb[c][:, tsl],
                            start=(c == ARR[0]),
                            stop=(c == ARR[-1]),
                            tile_position=(0, tp * 32),
                        )
                h_sb = mats.tile([128, 128], F32, tag="h")
                nc.vector.tensor_tensor(h_sb, kp, nine_sb, op=AluOpType.add)
                h_all.append(h_sb)

        # ---- stage 2: Newton-Schulz inverse, 4 groups pipelined -----------
        # X0 = c I(block) makes the first iteration closed-form:
        #   X1 = X0 (2I - H X0) = 2c I - c^2 H     (one DVE op, no matmuls)
        # The second (final) iteration runs in fp32 and squares the ~1%
        # linearization error away, so W~ is fp32-quality from one mm pair.
        wt_sb = []
        with tc.tile_pool(name="npsum", bufs=4, space="PSUM") as npsum:
            for g in range(G):
                x1 = mats.tile([128, 128], F32, tag="x1")
                nc.vector.scalar_tensor_tensor(
                    out=x1,
                    in0=h_all[g],
                    scalar=-NS_C * NS_C,
                    in1=cbd2_sb,
                    op0=AluOpType.mult,
                    op1=AluOpType.add,
                )
                t1p = npsum.tile([128, 128], F32, tag="t1p")
                nc.tensor.matmul(t1p, lhsT=h_all[g], rhs=x1, start=True, stop=True)
                u_ns = mats.tile([128, 128], F32, tag="u_ns")
                nc.vector.tensor_tensor(u_ns, i2_sb, t1p, op=AluOpType.subtract)
                x2p = npsum.tile([128, 128], F32, tag="x2p")
                nc.tensor.matmul(x2p, lhsT=x1, rhs=u_ns, start=True, stop=True)
                # bf16 W~: halves the per-ADMM-iteration weight reload on the
                # PE; error impact validated at +0.6e-3 on the logits
                wt = mats.tile([128, 128], BF16, tag="wt")
                nc.scalar.activation(
                    wt, x2p, mybir.ActivationFunctionType.Copy, scale=RHO
                )
                wt_sb.append(wt)

        # ---- stage 3: ADMM (d1/oy state form) -----------------------------
        # t = center(Wt @ d1) + y + OHC;  d1' = min(t, 2h-t);  oy' = max(t-(h-OHC), OHC)
        xb_sb = None
        # mpsum (ADMM) and cpsum (stage 4) coexist in time; both close before
        # stage 5 so lpsum can reuse their banks (lp waits on xb anyway).
        s34 = ExitStack()
        mpsum = s34.enter_context(tc.tile_pool(name="mpsum", bufs=2, space="PSUM"))
        for it in range(ADMM_ITERS):
            xp = mpsum.tile([128, 20], F32, tag="mp")
            for g in range(G):
                nc.tensor.matmul(
                    xp[:, g * NW : (g + 1) * NW],
                    lhsT=wt_sb[g],
                    rhs=d1_sb[:, g * NW : (g + 1) * NW],
                    start=True,
                    stop=True,
                )
            msum = state.tile([128, 4], F32, tag="msum")
            nc.vector.reduce_sum(
                msum,
                xp[:, :].rearrange("p (g w) -> p g w", w=NW),
                axis=mybir.AxisListType.X,
            )
            msb = msum[:, :]
            msb_ap = bass.AP(
                tensor=msb.tensor, offset=msb.offset, ap=[msb.ap[0], msb.ap[1], [0, NW]]
            )
            if it == ADMM_ITERS - 1:
                # last iteration only needs xb = center(Wt @ d1) in bf16
                xb_sb = state.tile([128, 20], BF16, tag="xb")
                nc.vector.scalar_tensor_tensor(
                    out=xb_sb[:, :].rearrange("p (g w) -> p g w", w=NW),
                    in0=msb_ap,
                    scalar=-1.0 / NW,
                    in1=xp[:, :].rearrange("p (g w) -> p g w", w=NW),
                    op0=AluOpType.mult,
                    op1=AluOpType.add,
                )
                break
            p1 = state.tile([128, 20], F32, tag="p1")
            nc.vector.tensor_tensor(p1, xp, oy_sb, op=AluOpType.add)
            tt_sb = state.tile([128, 20], F32, tag="tt")
            nc.vector.scalar_tensor_tensor(
                out=tt_sb[:, :].rearrange("p (g w) -> p g w", w=NW),
                in0=msb_ap,
                scalar=-1.0 / NW,
                in1=p1[:, :].rearrange("p (g w) -> p g w", w=NW),
                op0=AluOpType.mult,
                op1=AluOpType.add,
            )
            n2h = state.tile([128, 20], F32, tag="n2h")
            nc.vector.scalar_tensor_tensor(
                out=n2h,
                in0=tt_sb,
                scalar=-1.0,
                in1=h2_sb,
                op0=AluOpType.mult,
                op1=AluOpType.add,
            )
            d1_sb = state.tile([128, 20], BF16, tag="d1n")
            nc.vector.tensor_tensor(d1_sb, tt_sb, n2h, op=AluOpType.min)
            pa = state.tile([128, 20], F32, tag="pa")
            nc.vector.tensor_tensor(pa, tt_sb, hmo_sb, op=AluOpType.subtract)
            oy_sb = state.tile([128, 20], F32, tag="oy2")
            nc.vector.tensor_tensor(oy_sb, pa, ohc_sb, op=AluOpType.max)

        # ---- stage 4: epilogue C, hybrid orientation ----------------------
        # Group 3 keeps the direct s-major accumulation (one bank, four tp
        # bands = disjoint partitions): no evacuation or transpose, so its
        # cb copy lands ~13us, off the Act-queue tail. Groups 0-2 (12 tasks)
        # use the flipped C^T = Q S^T orientation — the 75-wide Q^T slice
        # rides as lhsT (weight loads overlapped/free) and only 25 S^T
        # columns stream per matmul, ~2.4x less PE time. All flipped chains
        # write partitions 0..74 and start=True clears the written partitions
        # across the WHOLE 2KB bank, so one chain per bank: three bank-waves
        # of <=5 (ctsum 5 + g3sum 1 + mpsum 2 = 8 banks; pool rotation makes
        # wave N+1 wait on wave N's evacuation). All emitted after the ADMM
        # so the scheduler drains C matmuls into the PE dead time between
        # ADMM iterations; evacuations ride the Act queue (gpsimd cannot
        # read PSUM, the DVE is mid-ADMM).
        cb_all = [None] * G
        g3sum = s34.enter_context(tc.tile_pool(name="g3sum", bufs=1, space="PSUM"))
        cp3 = g3sum.tile([128, NQ], F32, tag="cp3")
        for c in range(NCH):
            for tp in range(GP):
                t = 3 * GP + tp
                sl = slice(tp * 32, tp * 32 + NS)
                nc.tensor.matmul(
                    cp3[sl, :],
                    lhsT=st_sb[c][:, t * NS : (t + 1) * NS],
                    rhs=qt_sb[c][:, t * NQ : (t + 1) * NQ],
                    start=(c == 0),
                    stop=(c == NCH - 1),
                    tile_position=(0, tp * 32),
                )
        cb3 = wout.tile([128, NQ], BF16, tag="cb")
        nc.scalar.activation(cb3, cp3, mybir.ActivationFunctionType.Copy)
        cb_all[3] = cb3

        ctb_all = [None] * (3 * GP)
        with tc.tile_pool(name="ctsum", bufs=5, space="PSUM") as ctsum:
            for wave in (range(0, 5), range(5, 10), range(10, 12)):
                cts = {}
                for t in wave:
                    ct = ctsum.tile([NQ, NS], F32, tag="ct")
                    cts[t] = ct
                for c in range(NCH):
                    for t in wave:
                        nc.tensor.matmul(
                            cts[t],
                            lhsT=qt_sb[c][:, t * NQ : (t + 1) * NQ],
                            rhs=st_sb[c][:, t * NS : (t + 1) * NS],
                            start=(c == 0),
                            stop=(c == NCH - 1),
                        )
                # all on Act: gpsimd cannot read PSUM, and the DVE queue is
                # mid-ADMM (in-order — any insert would delay the chain)
                for t in wave:
                    ctb = ctbp.tile([NQ, NS], BF16, tag="ctb")
                    nc.scalar.activation(
                        ctb, cts[t], mybir.ActivationFunctionType.Copy
                    )
                    ctb_all[t] = ctb

        # C^T -> C via 12 single-shot identity matmuls (exact in bf16): the
        # transposed tasks land in the same 32-banded group layout the logits
        # stage already consumes. Disjoint-partition single-shot writes may
        # share a bank.
        cpsum = s34.enter_context(tc.tile_pool(name="cpsum", bufs=3, space="PSUM"))
        cp_all = []
        for g in range(3):
            cp = cpsum.tile([128, NQ], F32, tag="cp")
            cp_all.append(cp)
        for g in range(3):
            for tp in range(GP):
                t = g * GP + tp
                sl = slice(tp * 32, tp * 32 + NS)
                nc.tensor.matmul(
                    cp_all[g][sl, :],
                    lhsT=ctb_all[t],
                    rhs=i75b_sb,
                    start=True,
                    stop=True,
                    tile_position=(0, tp * 32),
                )

        # ---- stage 5: logits = C^T x, scaled ------------------------------
        # C evacuates PSUM on the Act engine (the DVE is mid-ADMM); the four
        # per-tp logit accumulations then land in four fresh banks (one chain
        # per bank, same-row-band matmuls serialize on their PE sub-array),
        # and four strided activations apply the scale and lay task t's
        # columns at out_sb[:, t*NW] in one op per tp band.
        for g in range(3):
            cb = wout.tile([128, NQ], BF16, tag="cb")
            if g % 2 == 0:
                nc.scalar.activation(
                    cb, cp_all[g], mybir.ActivationFunctionType.Copy
                )
            else:
                nc.vector.tensor_copy(cb, cp_all[g])
            cb_all[g] = cb
        s34.close()
        out_sb = consts.tile([NQ, T * NW], F32, tag="outsb")
        with tc.tile_pool(name="lpsum", bufs=GP, space="PSUM") as lpsum:
            for tp in range(GP):
                sl = slice(tp * 32, tp * 32 + NS)
                lp = lpsum.tile([NQ, G * NW], F32, tag="lp")
                for g in range(G):
                    nc.tensor.matmul(
                        lp[:, g * NW : (g + 1) * NW],
                        lhsT=cb_all[g][sl, :],
                        rhs=xb_sb[sl, g * NW : (g + 1) * NW],
                        start=(g == 0),
                        stop=(g == G - 1),
                        tile_position=(tp * 32, 0),
                    )
                ov = out_sb[:, tp * NW : tp * NW + NW]
                ov_ap = bass.AP(
                    tensor=ov.tensor,
                    offset=ov.offset,
                    ap=[ov.ap[0], [GP * NW, G], [1, NW]],
                )
                lp_view = lp[:, :].rearrange("q (g w) -> q g w", w=NW)
                if tp % 2 == 0:
                    nc.scalar.activation(
                        ov_ap,
                        lp_view,
                        mybir.ActivationFunctionType.Copy,
                        scale=scale_sb,
                    )
                else:
                    nc.vector.tensor_scalar_mul(ov_ap, lp_view, scale_sb[:, 0:1])
        nc.sync.dma_start(out=out_d[:, :], in_=out_sb)

    _split_waits(nc)
    return nc


_NC_CACHE = None


def _get_nc():
    global _NC_CACHE
    if _NC_CACHE is None:
        _NC_CACHE = _build_program()
    return _NC_CACHE


# ---------------------------------------------------------------------------
def _host_prep(support, query, support_labels, scale):
    """Shard + pack into the two DMA tensors. Layout only, no FLOPs."""
    f32 = np.float32
    bf = mybir.dt.np(BF16)
    eye = np.eye(NS, dtype=f32)
    blockdiag = np.zeros((128, 128), dtype=f32)
    for tp in range(GP):
        blockdiag[tp * 32 : tp * 32 + NS, tp * 32 : tp * 32 + NS] = eye

    in_maps = []
    for core in range(N_CORES):
        sl = slice(core * T, (core + 1) * T)
        S = np.asarray(support[sl], dtype=f32)        # [16,25,2560]
        Q = np.asarray(query[sl], dtype=f32)          # [16,75,2560]
        lab = np.asarray(support_labels[sl])          # [16,25] int
        data = np.empty((128, DATA_COLS), dtype=bf)
        # S^T chunks: col block c holds S[:, :, c*128:(c+1)*128]^T as [128, T*NS]
        data[:, ST0:QT0] = (
            S.transpose(2, 0, 1).reshape(NCH, 128, T * NS)
            .transpose(1, 0, 2).reshape(128, NCH * T * NS).astype(bf)
        )
        # Q^T chunks
        data[:, QT0:] = (
            Q.transpose(2, 0, 1).reshape(NCH, 128, T * NQ)
            .transpose(1, 0, 2).reshape(128, NCH * T * NQ).astype(bf)
        )
        oh = (lab[:, :, None] == np.arange(NW)[None, None, :]).astype(f32)
        # [16,25,5] -> [100,20]: row = tp*25+s, col = g*5+w
        ohm = np.zeros((128, 20), dtype=f32)
        ohr = oh.reshape(G, GP, NS, NW).transpose(1, 2, 0, 3).reshape(GP, NS, 20)
        for tp in range(GP):
            ohm[tp * 32 : tp * 32 + NS, :] = ohr[tp]
        cpack = np.zeros((128, CPACK_COLS), dtype=f32)
        cpack[:, OHC0 : OHC0 + 20] = ohm / RHO
        cpack[:, H20 : H20 + 20] = 2.0 * (C_REG + 1.0 / RHO) * ohm
        cpack[:, HMO0 : HMO0 + 20] = C_REG * ohm
        cpack[:, BD0 : BD0 + 128] = blockdiag
        cpack[0, SCL0] = np.asarray(scale, dtype=f32).reshape(-1)[0]
        cpack[0:NQ, I75_0 : I75_0 + NQ] = np.eye(NQ, dtype=f32)
        in_maps.append({"data": data, "cpack": cpack})
    return in_maps


def kernel(query, support, scale, support_labels, n_way, n_shot):
    assert int(n_way) == NW and int(n_shot) * int(n_way) == NS
    assert query.shape == (B_TOT, NQ, D) and support.shape == (B_TOT, NS, D)
    nc = _get_nc()
    in_maps = _host_prep(support, query, support_labels, scale)
    res = run_bass_kernel_spmd(nc, in_maps, core_ids=list(range(N_CORES)))
    outs = []
    for core in range(N_CORES):
        o = np.asarray(res.results[core]["out"])      # [75, 80]
        outs.append(o.reshape(NQ, T, NW).transpose(1, 0, 2))
    return np.ascontiguousarray(np.concatenate(outs, axis=0), dtype=np.float32)

